# revision 18
# baseline (speedup 1.0000x reference)
"""Causal self-attention (B=2, T=2048, C=1024, NH=16) on 8 trn2 NeuronCores.

Sharding: core c handles batch b = c//4 and head group g = c%4 (4 heads,
256 features). Each core computes q/k/v for its heads, causal attention in
S^T layout (keys on partitions, queries on the free dim), and a partial
output projection  y_heads @ w_proj[head_rows, :].  The host sums the four
partial projections per batch and adds b_proj.

v2 design (vs baseline):
  - All matmuls in bf16 (same PE rate as fp32r, halves SBUF/LDWEIGHTS/DMA).
    Weights are cast to bf16 host-side; q-scale 1/8 folded into wq/bq.
  - Phases interleaved per 512-query chunk (qc): transposes -> QKV(qc) ->
    attention(qc) -> projection(qc-1), keeping the in-order PE queue fed so
    the tensor engine stays at full p-state.
  - Attention inner loop software-pipelined: score matmuls for ki+2 are
    emitted before the PV matmuls of ki, hiding the exp (ACT) latency.
  - Both heads of a pair share one [128, 1024] score PSUM tile (2 banks);
    exp runs once per ki over both halves (halves ACT instruction count).
  - Softmax denominators via a ones-column in v_sb (row 64 of the PV psum);
    normalization: reciprocal_approx_fast -> gpsimd broadcast -> 2 muls.
  - QK bias fused into the PSUM->SBUF copy (tensor_scalar_add, per-partition
    bias AP); V bias as a rank-1 matmul accumulated into the V psum.
  - PSUM->SBUF copies spread over vector/gpsimd; exp exclusive on scalar.
"""

import os
import sys

import numpy as np

for _p in ("/opt/trn_rl_repo", "/root/.axon_site/_ro/trn_rl_repo"):
    if _p not in sys.path and os.path.isdir(_p):
        sys.path.append(_p)

import ml_dtypes  # noqa: E402

import concourse.bass as bass  # noqa: E402
import concourse.tile as tile  # noqa: E402
from concourse import bacc, mybir  # noqa: E402
from concourse.bass_utils import run_bass_kernel_spmd  # noqa: E402

P = 128
B, T, C = 2, 2048, 1024
NH, HD = 16, 64
FPC = 256  # features per core (4 heads)
QCW = 512  # query-chunk width
NCB = C // P  # 8 contraction blocks
NT = T // P  # 16 token tiles
NQC = T // QCW  # 4 query chunks
F32 = mybir.dt.float32
F32R = mybir.dt.float32r
BF16 = mybir.dt.bfloat16
EXPF = mybir.ActivationFunctionType.Exp


def build_nc(debug=False):
    nc = bacc.Bacc("TRN2", target_bir_lowering=False, debug=False)

    x_d = nc.dram_tensor("x", [T, C], F32R, kind="ExternalInput")
    wq_d = nc.dram_tensor("wq", [C, FPC], BF16, kind="ExternalInput")
    wk_d = nc.dram_tensor("wk", [C, FPC], BF16, kind="ExternalInput")
    wv_d = nc.dram_tensor("wv", [C, FPC], BF16, kind="ExternalInput")
    bqk_d = nc.dram_tensor("bqk", [2, FPC], F32, kind="ExternalInput")
    bv_d = nc.dram_tensor("bv", [1, FPC], BF16, kind="ExternalInput")
    wp_d = nc.dram_tensor("wp", [FPC, C], BF16, kind="ExternalInput")
    triu2_d = nc.dram_tensor("triu2", [P, 2 * P], BF16, kind="ExternalInput")
    ident_d = nc.dram_tensor("ident", [P, P], F32R, kind="ExternalInput")
    ones_d = nc.dram_tensor("ones", [1, P], BF16, kind="ExternalInput")
    out_d = nc.dram_tensor("out", [T, C], F32, kind="ExternalOutput")

    from contextlib import ExitStack

    with tile.TileContext(nc) as tc, ExitStack() as ctx:
        consts = ctx.enter_context(tc.tile_pool(name="consts", bufs=1))
        stage = ctx.enter_context(tc.tile_pool(name="stage", bufs=6))
        bigs = ctx.enter_context(tc.tile_pool(name="bigs", bufs=1))
        eapool = ctx.enter_context(tc.tile_pool(name="eapool", bufs=3))
        smalls = ctx.enter_context(tc.tile_pool(name="smalls", bufs=2))
        osts = ctx.enter_context(tc.tile_pool(name="osts", bufs=2))
        psum = ctx.enter_context(tc.tile_pool(name="psum", bufs=1, space="PSUM"))

        # ---- constants ----
        ident = consts.tile([P, P], F32R)
        nc.sync.dma_start(out=ident, in_=ident_d.ap())
        triu2 = consts.tile([P, 2 * P], BF16)
        nc.sync.dma_start(out=triu2, in_=triu2_d.ap())
        onesr = consts.tile([1, P], BF16)
        nc.sync.dma_start(out=onesr, in_=ones_d.ap())
        bqk = consts.tile([P, 2, 2], F32)  # [feat-in-pair, widx(q,k), pair]
        nc.sync.dma_start(out=bqk, in_=bqk_d.ap().rearrange("w (pr p) -> p w pr", p=P))
        bv = consts.tile([1, FPC], BF16)
        nc.sync.dma_start(out=bv, in_=bv_d.ap())

        # ---- persistent tiles ----
        xt = bigs.tile([P, NCB, T], BF16, tag="xt")  # X^T blocks
        qt = bigs.tile([P, 2, T], BF16, tag="qt")  # Q^T per pair (pre-scaled)
        kt = bigs.tile([P, 2, T], BF16, tag="kt")  # K^T per pair
        # V + ones col per head: [.., pair, head(64)+1+head(64)+1]
        v_sb = bigs.tile([P, NT, 2, 130], BF16, tag="v")
        yt = bigs.tile([P, 2, T], BF16, tag="yt")  # normalized y^T per pair
        wq_sb = bigs.tile([P, NCB, FPC], BF16, tag="wq")
        wk_sb = bigs.tile([P, NCB, FPC], BF16, tag="wk")
        wv_sb = bigs.tile([P, NCB, FPC], BF16, tag="wv")
        wp_sb = bigs.tile([P, 2, C], BF16, tag="wp")

        # ones columns (64 and 129) of v_sb give softmax denominators in
        # row 64/129 of the PV psum.
        nc.vector.memset(v_sb[:, :, :, 64::65], 1.0)

        # ---- x tile loads (3-deep prefetch) + weights ----
        xst = [None] * NT

        def load_x(t):
            s = stage.tile([P, C], F32R, tag="xs", name=f"xs{t}")
            nc.sync.dma_start(out=s, in_=x_d.ap()[t * P : (t + 1) * P, :])
            xst[t] = s

        for t in range(5):
            load_x(t)
        for wsb, wd in ((wq_sb, wq_d), (wk_sb, wk_d), (wv_sb, wv_d)):
            nc.sync.dma_start(out=wsb, in_=wd.ap().rearrange("(cb p) f -> p cb f", p=P))
        nc.sync.dma_start(out=wp_sb, in_=wp_d.ap().rearrange("(fb p) o -> p fb o", p=P))

        def transpose_tile(t):
            ps = psum.tile([P, 2 * QCW], F32R, tag="st", name="tp", bufs=2)
            for cb in range(NCB):
                nc.tensor.transpose(
                    ps[:, cb * P : (cb + 1) * P],
                    xst[t][:, cb * P : (cb + 1) * P],
                    ident,
                )
            dst = xt[:, :, t * P : (t + 1) * P]
            src = ps.rearrange("p (c w) -> p c w", c=NCB)
            nc.vector.tensor_copy(out=dst, in_=src)

        def qkv(qc):
            cs = slice(qc * QCW, (qc + 1) * QCW)
            for pair in range(2):
                fs = slice(pair * P, (pair + 1) * P)
                for widx, wsb, dstt in ((1, wk_sb, kt), (0, wq_sb, qt)):
                    ps = psum.tile([P, QCW], F32, tag="mm", name="qk", bufs=2)
                    for cb in range(NCB):
                        nc.tensor.matmul(
                            ps,
                            wsb[:, cb, fs],
                            xt[:, cb, cs],
                            start=(cb == 0),
                            stop=(cb == NCB - 1),
                        )
                    nc.vector.tensor_scalar_add(
                        dstt[:, pair, cs], ps, bqk[:, widx, pair : pair + 1]
                    )
            for t in range(qc * 4, qc * 4 + 4):
                ps = psum.tile([P, FPC], F32, tag="mm", name="vv", bufs=2)
                nc.tensor.matmul(ps, onesr[0:1, :], bv[0:1, :], start=True, stop=False)
                for cb in range(NCB):
                    nc.tensor.matmul(
                        ps,
                        xt[:, cb, t * P : (t + 1) * P],
                        wv_sb[:, cb, :],
                        start=False,
                        stop=(cb == NCB - 1),
                    )
                nc.vector.tensor_copy(
                    out=v_sb[:, t].rearrange("p a (h w) -> p a h w", w=65)[
                        :, :, :, 0:64
                    ],
                    in_=ps.rearrange("p (a h w) -> p a h w", a=2, w=64),
                )

        def attn(pair, qc):
            nki = 4 * (qc + 1)
            cs0 = qc * QCW
            # diagonal k-tiles first: their exp->mask chain hides under the
            # full tiles, and the chunk tail is a clean exp->PV chain.
            ki_list = list(range(4 * qc, nki)) + list(range(0, 4 * qc))
            sts = []

            def emit_st(ki):
                m = ki - 4 * qc
                lo = max(m, 0) * P
                st = psum.tile([P, 2 * QCW], F32, tag="st", name="st", bufs=2)
                ks = slice(ki * P, (ki + 1) * P)
                nc.tensor.matmul(
                    st[0:P, lo:QCW],
                    kt[0:64, pair, ks],
                    qt[0:64, pair, cs0 + lo : cs0 + QCW],
                    start=True,
                    stop=True,
                )
                nc.tensor.matmul(
                    st[0:P, QCW + lo : 2 * QCW],
                    kt[64:P, pair, ks],
                    qt[64:P, pair, cs0 + lo : cs0 + QCW],
                    start=True,
                    stop=True,
                    tile_position=(64, 0),
                )
                sts.append((st, lo))

            yab = psum.tile([P, 2 * QCW], F32, tag="y", name="yab", bufs=1)
            emit_st(ki_list[0])
            if nki > 1:
                emit_st(ki_list[1])
            for idx, ki in enumerate(ki_list):
                st, lo = sts[idx]
                m = ki - 4 * qc
                eab = eapool.tile([P, 2 * QCW], BF16, tag="e", name="eab")
                stv = st.rearrange("p (h n) -> p h n", h=2)[:, :, lo:]
                eabv = eab.rearrange("p (h n) -> p h n", h=2)[:, :, lo:]
                nc.scalar.activation(eabv, stv, EXPF)
                if m >= 0:  # diagonal 128-block: causal triangle mask
                    dv = eab.rearrange("p (h n) -> p h n", h=2)[
                        :, :, m * P : (m + 1) * P
                    ]
                    tv = triu2.rearrange("p (h n) -> p h n", h=2)
                    nc.gpsimd.tensor_mul(dv, dv, tv)
                if idx + 2 < nki:
                    emit_st(ki_list[idx + 2])
                last = idx == nki - 1
                nc.tensor.matmul(
                    yab[0:65, lo:QCW],
                    v_sb[:, ki, pair, 0:65],
                    eab[:, lo:QCW],
                    start=(idx == 0),
                    stop=last,
                )
                nc.tensor.matmul(
                    yab[0:65, QCW + lo : 2 * QCW],
                    v_sb[:, ki, pair, 65:130],
                    eab[:, QCW + lo : 2 * QCW],
                    start=(idx == 0),
                    stop=last,
                )
            # evacuate yab at once so the PSUM banks free for the next chunk:
            # values via DVE, denominator row via ACT (to a partition-0 tile,
            # which reciprocal_approx_fast requires), in parallel.
            yu = smalls.tile([64, 2 * QCW], F32, tag="yu", name="yu")
            nc.vector.tensor_copy(out=yu, in_=yab[0:64, :])
            srow = smalls.tile([1, 2 * QCW], F32, tag="srow", name="srow")
            nc.scalar.activation(
                srow, yab[64:65, :], mybir.ActivationFunctionType.Copy
            )
            if debug and pair == 0 and qc == 0:
                d = nc.dram_tensor("dbg_yab", [P, 2 * QCW], F32, kind="ExternalOutput")
                nc.sync.dma_start(out=d.ap()[0:64, :], in_=yu)
                nc.sync.dma_start(out=d.ap()[64:65, :], in_=srow)
            # normalization tail
            rrow = smalls.tile([1, 2 * QCW], F32, tag="rrow", name="rrow")
            nc.vector.reciprocal_approx_fast(out=rrow, in_=srow)
            rb = smalls.tile([64, 2 * QCW], F32, tag="rb", name="rb")
            nc.gpsimd.dma_start(
                out=rb, in_=rrow[0:1, None, :].broadcast_to([1, 64, 2 * QCW])
            )
            nc.vector.tensor_mul(
                yt[0:64, pair, cs0 : cs0 + QCW], yu[0:64, 0:QCW], rb[:, 0:QCW]
            )
            nc.vector.tensor_mul(
                yt[64:P, pair, cs0 : cs0 + QCW],
                yu[0:64, QCW : 2 * QCW],
                rb[:, QCW : 2 * QCW],
            )

        def proj(qc):
            for t in range(qc * 4, qc * 4 + 4):
                ost = osts.tile([P, C], F32, tag="ost", name="ost")
                for ch in range(2):
                    ps = psum.tile([P, QCW], F32, tag="mm", name="pj", bufs=2)
                    for fb in range(2):
                        nc.tensor.matmul(
                            ps,
                            yt[:, fb, t * P : (t + 1) * P],
                            wp_sb[:, fb, ch * QCW : (ch + 1) * QCW],
                            start=(fb == 0),
                            stop=(fb == 1),
                        )
                    nc.vector.tensor_copy(
                        out=ost[:, ch * QCW : (ch + 1) * QCW], in_=ps
                    )
                nc.sync.dma_start(out=out_d.ap()[t * P : (t + 1) * P, :], in_=ost)

        # ---- main interleaved schedule ----
        for qc in range(NQC):
            for t in range(qc * 4, qc * 4 + 4):
                if t + 5 < NT:
                    load_x(t + 5)
                transpose_tile(t)
            qkv(qc)
            for pair in range(2):
                attn(pair, qc)
            if qc >= 1:
                proj(qc - 1)
        proj(NQC - 1)

        if debug:
            for nm, src in (
                ("dbg_xt", xt.rearrange("p a b -> p (a b)")),
                ("dbg_qt", qt.rearrange("p a b -> p (a b)")),
                ("dbg_kt", kt.rearrange("p a b -> p (a b)")),
                ("dbg_v", v_sb.rearrange("p a b c -> p (a b c)")),
                ("dbg_yt", yt.rearrange("p a b -> p (a b)")),
            ):
                d = nc.dram_tensor(nm, [P, src.free_size()], src.dtype, kind="ExternalOutput")
                nc.sync.dma_start(out=d.ap(), in_=src)

    nc.compile()
    return nc


_NC_CACHE: dict = {}
LAST_RESULT = None


def kernel(x, w_attn, b_attn, w_proj, b_proj):
    global LAST_RESULT
    bf = ml_dtypes.bfloat16
    x = np.ascontiguousarray(np.asarray(x, np.float32))
    w_attn = np.asarray(w_attn, np.float32)
    b_attn = np.asarray(b_attn, np.float32)
    w_proj = np.asarray(w_proj, np.float32)
    b_proj = np.asarray(b_proj, np.float32)

    if "nc" not in _NC_CACHE:
        _NC_CACHE["nc"] = build_nc()
    nc = _NC_CACHE["nc"]

    tri = np.triu(np.ones((P, P), np.float32))
    triu2 = np.ascontiguousarray(np.concatenate([tri, tri], axis=1)).astype(bf)
    ident = np.eye(P, dtype=np.float32)
    ones = np.ones((1, P), np.float32).astype(bf)

    in_maps = []
    for core in range(8):
        b, g = core // 4, core % 4
        f0 = g * FPC
        in_maps.append(
            {
                "x": np.ascontiguousarray(x[b]),
                "wq": np.ascontiguousarray(w_attn[:, f0 : f0 + FPC] * 0.125).astype(
                    bf
                ),
                "wk": np.ascontiguousarray(w_attn[:, C + f0 : C + f0 + FPC]).astype(
                    bf
                ),
                "wv": np.ascontiguousarray(
                    w_attn[:, 2 * C + f0 : 2 * C + f0 + FPC]
                ).astype(bf),
                "bqk": np.ascontiguousarray(
                    np.stack(
                        [
                            b_attn[f0 : f0 + FPC] * 0.125,
                            b_attn[C + f0 : C + f0 + FPC],
                        ]
                    )
                ).astype(np.float32),
                "bv": np.ascontiguousarray(
                    b_attn[2 * C + f0 : 2 * C + f0 + FPC].reshape(1, FPC)
                ).astype(bf),
                "wp": np.ascontiguousarray(w_proj[f0 : f0 + FPC, :]).astype(bf),
                "triu2": triu2,
                "ident": ident,
                "ones": ones,
            }
        )

    trace = bool(os.environ.get("BASS_TRACE"))
    res = run_bass_kernel_spmd(
        nc,
        in_maps,
        core_ids=list(range(8)),
        trace=trace,
        tmpdir=os.environ.get("KERNEL_TRACE_DIR") or None,
    )
    LAST_RESULT = res

    y = np.empty((B, T, C), np.float32)
    for b in range(B):
        acc = res.results[4 * b]["out"].astype(np.float32).copy()
        for g in range(1, 4):
            acc += res.results[4 * b + g]["out"]
        y[b] = acc + b_proj[None, :]
    return y


# revision 22
# speedup vs baseline: 1.1840x; 1.1840x over previous
"""Causal self-attention (B=2, T=2048, C=1024, NH=16) on 8 trn2 NeuronCores.

Sharding: core c handles batch b = c//4 and head group g = c%4 (4 heads,
256 features). Each core computes q/k/v for its heads, causal attention in
S^T layout (keys on partitions, queries on the free dim), and a partial
output projection  y_heads @ w_proj[head_rows, :].  The host sums the four
partial projections per batch and adds b_proj.

v2 design (vs baseline):
  - All matmuls in bf16 (same PE rate as fp32r, halves SBUF/LDWEIGHTS/DMA).
    Weights are cast to bf16 host-side; q-scale 1/8 folded into wq/bq.
  - Phases interleaved per 512-query chunk (qc): transposes -> QKV(qc) ->
    attention(qc) -> projection(qc-1), keeping the in-order PE queue fed so
    the tensor engine stays at full p-state.
  - Attention inner loop software-pipelined: score matmuls for ki+2 are
    emitted before the PV matmuls of ki, hiding the exp (ACT) latency.
  - Both heads of a pair share one [128, 1024] score PSUM tile (2 banks);
    exp runs once per ki over both halves (halves ACT instruction count).
  - Softmax denominators via a ones-column in v_sb (row 64 of the PV psum);
    normalization: reciprocal_approx_fast -> gpsimd broadcast -> 2 muls.
  - QK bias fused into the PSUM->SBUF copy (tensor_scalar_add, per-partition
    bias AP); V bias as a rank-1 matmul accumulated into the V psum.
  - PSUM->SBUF copies spread over vector/gpsimd; exp exclusive on scalar.
"""

import os
import sys

import numpy as np

for _p in ("/opt/trn_rl_repo", "/root/.axon_site/_ro/trn_rl_repo"):
    if _p not in sys.path and os.path.isdir(_p):
        sys.path.append(_p)

import ml_dtypes  # noqa: E402

import concourse.bass as bass  # noqa: E402
import concourse.tile as tile  # noqa: E402
from concourse import bacc, mybir  # noqa: E402
from concourse.bass_utils import run_bass_kernel_spmd  # noqa: E402

P = 128
B, T, C = 2, 2048, 1024
NH, HD = 16, 64
FPC = 256  # features per core (4 heads)
QCW = 512  # query-chunk width
NCB = C // P  # 8 contraction blocks
NT = T // P  # 16 token tiles
NQC = T // QCW  # 4 query chunks
F32 = mybir.dt.float32
F32R = mybir.dt.float32r
BF16 = mybir.dt.bfloat16
EXPF = mybir.ActivationFunctionType.Exp


def build_nc(debug=False):
    nc = bacc.Bacc("TRN2", target_bir_lowering=False, debug=False)

    x_d = nc.dram_tensor("x", [T, C], F32R, kind="ExternalInput")
    wq_d = nc.dram_tensor("wq", [C, FPC], BF16, kind="ExternalInput")
    wk_d = nc.dram_tensor("wk", [C, FPC], BF16, kind="ExternalInput")
    wv_d = nc.dram_tensor("wv", [C, FPC], BF16, kind="ExternalInput")
    bqk_d = nc.dram_tensor("bqk", [2, FPC], F32, kind="ExternalInput")
    bv_d = nc.dram_tensor("bv", [1, FPC], BF16, kind="ExternalInput")
    wp_d = nc.dram_tensor("wp", [FPC, C], BF16, kind="ExternalInput")
    triu2_d = nc.dram_tensor("triu2", [P, 2 * P], BF16, kind="ExternalInput")
    ident_d = nc.dram_tensor("ident", [P, P], F32R, kind="ExternalInput")
    ones_d = nc.dram_tensor("ones", [1, P], BF16, kind="ExternalInput")
    out_d = nc.dram_tensor("out", [T, C], F32, kind="ExternalOutput")

    from contextlib import ExitStack

    with tile.TileContext(nc) as tc, ExitStack() as ctx:
        consts = ctx.enter_context(tc.tile_pool(name="consts", bufs=1))
        stage = ctx.enter_context(tc.tile_pool(name="stage", bufs=6))
        bigs = ctx.enter_context(tc.tile_pool(name="bigs", bufs=1))
        eapool = ctx.enter_context(tc.tile_pool(name="eapool", bufs=3))
        smalls = ctx.enter_context(tc.tile_pool(name="smalls", bufs=2))
        osts = ctx.enter_context(tc.tile_pool(name="osts", bufs=2))
        psum = ctx.enter_context(tc.tile_pool(name="psum", bufs=1, space="PSUM"))

        # ---- constants ----
        ident = consts.tile([P, P], F32R)
        nc.sync.dma_start(out=ident, in_=ident_d.ap())
        triu2 = consts.tile([P, 2 * P], BF16)
        nc.sync.dma_start(out=triu2, in_=triu2_d.ap())
        onesr = consts.tile([1, P], BF16)
        nc.sync.dma_start(out=onesr, in_=ones_d.ap())
        bqk = consts.tile([P, 2, 2], F32)  # [feat-in-pair, widx(q,k), pair]
        nc.sync.dma_start(out=bqk, in_=bqk_d.ap().rearrange("w (pr p) -> p w pr", p=P))
        bv = consts.tile([1, FPC], BF16)
        nc.sync.dma_start(out=bv, in_=bv_d.ap())

        # ---- persistent tiles ----
        xt = bigs.tile([P, NCB, T], BF16, tag="xt")  # X^T blocks
        qt = bigs.tile([P, 2, T], BF16, tag="qt")  # Q^T per pair (pre-scaled)
        kt = bigs.tile([P, 2, T], BF16, tag="kt")  # K^T per pair
        # V + ones col per head: [.., pair, head(64)+1+head(64)+1]
        v_sb = bigs.tile([P, NT, 2, 130], BF16, tag="v")
        yt = bigs.tile([P, 2, T], BF16, tag="yt")  # normalized y^T per pair
        wq_sb = bigs.tile([P, NCB, FPC], BF16, tag="wq")
        wk_sb = bigs.tile([P, NCB, FPC], BF16, tag="wk")
        wv_sb = bigs.tile([P, NCB, FPC], BF16, tag="wv")
        wp_sb = bigs.tile([P, 2, C], BF16, tag="wp")

        # ones columns (64 and 129) of v_sb give softmax denominators in
        # row 64/129 of the PV psum.
        nc.vector.memset(v_sb[:, :, :, 64::65], 1.0)

        # ---- x tile loads (3-deep prefetch) + weights ----
        xst = [None] * NT

        def load_x(t):
            s = stage.tile([P, C], F32R, tag="xs", name=f"xs{t}")
            nc.sync.dma_start(out=s, in_=x_d.ap()[t * P : (t + 1) * P, :])
            xst[t] = s

        for t in range(5):
            load_x(t)
        for wsb, wd in ((wq_sb, wq_d), (wk_sb, wk_d), (wv_sb, wv_d)):
            nc.sync.dma_start(out=wsb, in_=wd.ap().rearrange("(cb p) f -> p cb f", p=P))
        nc.sync.dma_start(out=wp_sb, in_=wp_d.ap().rearrange("(fb p) o -> p fb o", p=P))

        def transpose_tile(t):
            ps = psum.tile([P, 2 * QCW], F32R, tag="st", name="tp", bufs=2)
            for cb in range(NCB):
                nc.tensor.transpose(
                    ps[:, cb * P : (cb + 1) * P],
                    xst[t][:, cb * P : (cb + 1) * P],
                    ident,
                )
            dst = xt[:, :, t * P : (t + 1) * P]
            src = ps.rearrange("p (c w) -> p c w", c=NCB)
            nc.vector.tensor_copy(out=dst, in_=src)

        def qkv(qc):
            cs = slice(qc * QCW, (qc + 1) * QCW)
            for pair in range(2):
                fs = slice(pair * P, (pair + 1) * P)
                for widx, wsb, dstt in ((1, wk_sb, kt), (0, wq_sb, qt)):
                    ps = psum.tile([P, QCW], F32, tag="mm", name="qk", bufs=2)
                    for cb in range(NCB):
                        nc.tensor.matmul(
                            ps,
                            wsb[:, cb, fs],
                            xt[:, cb, cs],
                            start=(cb == 0),
                            stop=(cb == NCB - 1),
                        )
                    nc.vector.tensor_scalar_add(
                        dstt[:, pair, cs], ps, bqk[:, widx, pair : pair + 1]
                    )
            for t in range(qc * 4, qc * 4 + 4):
                ps = psum.tile([P, FPC], F32, tag="mm", name="vv", bufs=2)
                nc.tensor.matmul(ps, onesr[0:1, :], bv[0:1, :], start=True, stop=False)
                for cb in range(NCB):
                    nc.tensor.matmul(
                        ps,
                        xt[:, cb, t * P : (t + 1) * P],
                        wv_sb[:, cb, :],
                        start=False,
                        stop=(cb == NCB - 1),
                    )
                nc.vector.tensor_copy(
                    out=v_sb[:, t].rearrange("p a (h w) -> p a h w", w=65)[
                        :, :, :, 0:64
                    ],
                    in_=ps.rearrange("p (a h w) -> p a h w", a=2, w=64),
                )

        pending_sts = {}

        def emit_st(pair, qc, ki, sts):
            cs0 = qc * QCW
            m = ki - 4 * qc
            lo = max(m, 0) * P
            st = psum.tile([P, 2 * QCW], F32, tag="st", name="st", bufs=2)
            ks = slice(ki * P, (ki + 1) * P)
            nc.tensor.matmul(
                st[0:P, lo:QCW],
                kt[0:64, pair, ks],
                qt[0:64, pair, cs0 + lo : cs0 + QCW],
                start=True,
                stop=True,
            )
            nc.tensor.matmul(
                st[0:P, QCW + lo : 2 * QCW],
                kt[64:P, pair, ks],
                qt[64:P, pair, cs0 + lo : cs0 + QCW],
                start=True,
                stop=True,
                tile_position=(64, 0),
            )
            sts.append((st, lo))

        def attn(pair, qc, prefix_fn=None):
            nki = 4 * (qc + 1)
            cs0 = qc * QCW
            # diagonal k-tiles first: their exp->mask chain hides under the
            # full tiles, and the chunk tail is a clean exp->PV chain.
            ki_list = list(range(4 * qc, nki)) + list(range(0, 4 * qc))
            sts = pending_sts.pop((pair, qc), [])

            yab = psum.tile([P, 2 * QCW], F32, tag="y", name="yab", bufs=1)
            while len(sts) < min(2, nki):
                emit_st(pair, qc, ki_list[len(sts)], sts)
            for idx, ki in enumerate(ki_list):
                st, lo = sts[idx]
                m = ki - 4 * qc
                eab = eapool.tile([P, 2 * QCW], BF16, tag="e", name="eab")
                stv = st.rearrange("p (h n) -> p h n", h=2)[:, :, lo:]
                eabv = eab.rearrange("p (h n) -> p h n", h=2)[:, :, lo:]
                nc.scalar.activation(eabv, stv, EXPF)
                if m >= 0:  # diagonal 128-block: causal triangle mask
                    dv = eab.rearrange("p (h n) -> p h n", h=2)[
                        :, :, m * P : (m + 1) * P
                    ]
                    tv = triu2.rearrange("p (h n) -> p h n", h=2)
                    nc.gpsimd.tensor_mul(dv, dv, tv)
                if idx + 2 < nki:
                    emit_st(pair, qc, ki_list[idx + 2], sts)
                last = idx == nki - 1
                nc.tensor.matmul(
                    yab[0:65, lo:QCW],
                    v_sb[:, ki, pair, 0:65],
                    eab[:, lo:QCW],
                    start=(idx == 0),
                    stop=last,
                )
                nc.tensor.matmul(
                    yab[0:65, QCW + lo : 2 * QCW],
                    v_sb[:, ki, pair, 65:130],
                    eab[:, QCW + lo : 2 * QCW],
                    start=(idx == 0),
                    stop=last,
                )
            # evacuate yab at once so the PSUM banks free for the next chunk:
            # denominator row via ACT (to a partition-0 tile, which
            # reciprocal_approx_fast requires), values via DVE, in parallel.
            srow = smalls.tile([1, 2 * QCW], F32, tag="srow", name="srow")
            nc.scalar.activation(
                srow, yab[64:65, :], mybir.ActivationFunctionType.Copy
            )
            yu = smalls.tile([64, 2 * QCW], F32, tag="yu", name="yu")
            nc.vector.tensor_copy(out=yu, in_=yab[0:64, :])
            if debug and pair == 0 and qc == 0:
                d = nc.dram_tensor("dbg_yab", [P, 2 * QCW], F32, kind="ExternalOutput")
                nc.sync.dma_start(out=d.ap()[0:64, :], in_=yu)
                nc.sync.dma_start(out=d.ap()[64:65, :], in_=srow)
            rrow = smalls.tile([1, 2 * QCW], F32, tag="rrow", name="rrow")
            nc.vector.reciprocal_approx_fast(out=rrow, in_=srow)
            # PE work emitted by the caller to cover the recip latency
            # (next chunk's first score tiles, or the previous projection).
            if prefix_fn is not None:
                prefix_fn()
            # broadcast 1/denom to 64 partitions with two rank-1 matmuls
            # (a gpsimd SWDGE broadcast DMA lands ~10us late behind bulk DMA
            # traffic; the PE does it in 250ns each).
            rrbf = smalls.tile([1, 2 * QCW], BF16, tag="rrbf", name="rrbf")
            nc.vector.tensor_copy(out=rrbf, in_=rrow)
            rbs = []
            for h in range(2):
                rb = psum.tile([64, QCW], F32, tag="mm", name="rb", bufs=2)
                nc.tensor.matmul(
                    rb,
                    onesr[0:1, 0:64],
                    rrbf[0:1, h * QCW : (h + 1) * QCW],
                    start=True,
                    stop=True,
                )
                rbs.append(rb)
            nc.vector.tensor_mul(
                yt[0:64, pair, cs0 : cs0 + QCW], yu[0:64, 0:QCW], rbs[0]
            )
            nc.vector.tensor_mul(
                yt[64:P, pair, cs0 : cs0 + QCW],
                yu[0:64, QCW : 2 * QCW],
                rbs[1],
            )

        def proj(qc):
            for t in range(qc * 4, qc * 4 + 4):
                ost = osts.tile([P, C], F32, tag="ost", name="ost")
                for ch in range(2):
                    ps = psum.tile([P, QCW], F32, tag="mm", name="pj", bufs=2)
                    for fb in range(2):
                        nc.tensor.matmul(
                            ps,
                            yt[:, fb, t * P : (t + 1) * P],
                            wp_sb[:, fb, ch * QCW : (ch + 1) * QCW],
                            start=(fb == 0),
                            stop=(fb == 1),
                        )
                    nc.vector.tensor_copy(
                        out=ost[:, ch * QCW : (ch + 1) * QCW], in_=ps
                    )
                nc.sync.dma_start(out=out_d.ap()[t * P : (t + 1) * P, :], in_=ost)

        # ---- main interleaved schedule ----
        for qc in range(NQC):
            for t in range(qc * 4, qc * 4 + 4):
                if t + 5 < NT:
                    load_x(t + 5)
                transpose_tile(t)
            qkv(qc)

            def pre_pair1(qc=qc):
                sts = []
                nki = 4 * (qc + 1)
                ki_list = list(range(4 * qc, nki)) + list(range(0, 4 * qc))
                for i in range(min(2, nki)):
                    emit_st(1, qc, ki_list[i], sts)
                pending_sts[(1, qc)] = sts

            attn(0, qc, prefix_fn=pre_pair1)
            attn(1, qc, prefix_fn=(lambda qc=qc: proj(qc - 1)) if qc >= 1 else None)
        proj(NQC - 1)

        if debug:
            for nm, src in (
                ("dbg_xt", xt.rearrange("p a b -> p (a b)")),
                ("dbg_qt", qt.rearrange("p a b -> p (a b)")),
                ("dbg_kt", kt.rearrange("p a b -> p (a b)")),
                ("dbg_v", v_sb.rearrange("p a b c -> p (a b c)")),
                ("dbg_yt", yt.rearrange("p a b -> p (a b)")),
            ):
                d = nc.dram_tensor(nm, [P, src.free_size()], src.dtype, kind="ExternalOutput")
                nc.sync.dma_start(out=d.ap(), in_=src)

    nc.compile()
    return nc


_NC_CACHE: dict = {}
LAST_RESULT = None


def kernel(x, w_attn, b_attn, w_proj, b_proj):
    global LAST_RESULT
    bf = ml_dtypes.bfloat16
    x = np.ascontiguousarray(np.asarray(x, np.float32))
    w_attn = np.asarray(w_attn, np.float32)
    b_attn = np.asarray(b_attn, np.float32)
    w_proj = np.asarray(w_proj, np.float32)
    b_proj = np.asarray(b_proj, np.float32)

    if "nc" not in _NC_CACHE:
        _NC_CACHE["nc"] = build_nc()
    nc = _NC_CACHE["nc"]

    tri = np.triu(np.ones((P, P), np.float32))
    triu2 = np.ascontiguousarray(np.concatenate([tri, tri], axis=1)).astype(bf)
    ident = np.eye(P, dtype=np.float32)
    ones = np.ones((1, P), np.float32).astype(bf)

    in_maps = []
    for core in range(8):
        b, g = core // 4, core % 4
        f0 = g * FPC
        in_maps.append(
            {
                "x": np.ascontiguousarray(x[b]),
                "wq": np.ascontiguousarray(w_attn[:, f0 : f0 + FPC] * 0.125).astype(
                    bf
                ),
                "wk": np.ascontiguousarray(w_attn[:, C + f0 : C + f0 + FPC]).astype(
                    bf
                ),
                "wv": np.ascontiguousarray(
                    w_attn[:, 2 * C + f0 : 2 * C + f0 + FPC]
                ).astype(bf),
                "bqk": np.ascontiguousarray(
                    np.stack(
                        [
                            b_attn[f0 : f0 + FPC] * 0.125,
                            b_attn[C + f0 : C + f0 + FPC],
                        ]
                    )
                ).astype(np.float32),
                "bv": np.ascontiguousarray(
                    b_attn[2 * C + f0 : 2 * C + f0 + FPC].reshape(1, FPC)
                ).astype(bf),
                "wp": np.ascontiguousarray(w_proj[f0 : f0 + FPC, :]).astype(bf),
                "triu2": triu2,
                "ident": ident,
                "ones": ones,
            }
        )

    trace = bool(os.environ.get("BASS_TRACE"))
    res = run_bass_kernel_spmd(
        nc,
        in_maps,
        core_ids=list(range(8)),
        trace=trace,
        tmpdir=os.environ.get("KERNEL_TRACE_DIR") or None,
    )
    LAST_RESULT = res

    y = np.empty((B, T, C), np.float32)
    for b in range(B):
        acc = res.results[4 * b]["out"].astype(np.float32).copy()
        for g in range(1, 4):
            acc += res.results[4 * b + g]["out"]
        y[b] = acc + b_proj[None, :]
    return y
